# revision 9
# baseline (speedup 1.0000x reference)
"""Causal multi-head attention (fused QKV) on 8 Trainium2 NeuronCores.

Problem: x[2, 2048, 1024] @ W_qkv[1024, 3072] -> causal MHA, 16 heads,
head_dim 64 -> out [2, 2048, 1024].

Sharding: batch (2) x head-groups (4) = 8 shards; core c handles batch
c//4, heads 4*(c%4) .. 4*(c%4)+3.  Each core is fully independent (no
collectives).

Per-core layouts (host prepares):
  xT   [D, S]      x[b].T  (contraction dim D on partitions)
  w    [D, 768]    W columns reordered [Q(4x64) | K(4x64) | V(4x64)]
  qkb  [128, 4]    QK bias per 128-row chunk (column fc)
  vb   [128, 256]  V bias replicated across partitions
  outT [256, S]    output transposed: row 64*h+j, col s = out[b,s,h,j]

On-core algorithm (all matmuls fp32r = full-rate fp32-reduced):
  QK.T = w_qk.T @ x.T   -> [512, S] rows (Q_h0|Q_h1|Q_h2|Q_h3|K_h0..)
  V    = x @ w_v        -> [S, 256], stored as [V_h|1] blocks of 65 cols
  per head pair, per q-chunk (512), per k-block (128, causal-pruned):
    S_T[k,q] = K@Q.T via 2 row-tiled (K=64) concurrent matmuls
    P = exp(S_T/8) on ScalarE (PSUM->SBUF, valid q range only)
    diagonal k-block: P *= upper-tri mask (DVE)
    out.T[hd+1, q] += [V|1].T @ P  (65th row accumulates softmax denom)
  normalize: r = 1/denom (DVE), partition-broadcast (GPSIMD), multiply.
"""

import sys

if "/opt/trn_rl_repo" not in sys.path:
    sys.path.insert(0, "/opt/trn_rl_repo")

import numpy as np

import concourse.bass as bass
import concourse.mybir as mybir
import concourse.tile as tile
from concourse import bacc
from concourse.bass_utils import run_bass_kernel_spmd
from concourse.masks import make_upper_triangular

F32 = mybir.dt.float32
F32R = mybir.dt.float32r
EXP = mybir.ActivationFunctionType.Exp
MULT = mybir.AluOpType.mult
ADD = mybir.AluOpType.add

N_CORES = 8
B, S, D = 2, 2048, 1024
N_HEAD = 16
HD = 64  # head dim
HPC = 4  # heads per core
FQK = 2 * HPC * HD  # 512 rows of QK.T
FV = HPC * HD  # 256 cols of V


def build_mha_core(S=S, D=D, trace_sim=False):
    """Build the per-core Bass program. Returns (nc, input names)."""
    NQC = S // 512  # 512-wide q chunks
    NKB = S // 128  # 128-wide k blocks
    NDC = D // 128  # 128-deep contraction chunks
    VW = HD + 1  # V block width incl. ones column

    nc = bacc.Bacc("TRN2", target_bir_lowering=False, debug=False)
    xT_d = nc.dram_tensor("xT", [D, S], F32R, kind="ExternalInput")
    w_d = nc.dram_tensor("w", [D, 768], F32R, kind="ExternalInput")
    qkb_d = nc.dram_tensor("qkb", [128, 4], F32, kind="ExternalInput")
    ones_d = nc.dram_tensor("ones", [128, S // 128 * HPC], F32R, kind="ExternalInput")
    vb_d = nc.dram_tensor("vb", [128, FV], F32, kind="ExternalInput")
    outT_d = nc.dram_tensor("outT", [FV, S], F32, kind="ExternalOutput")

    with tile.TileContext(nc, trace_sim=trace_sim) as tc:
        with (
            tc.tile_pool(name="const", bufs=1) as const,
            tc.tile_pool(name="big", bufs=1) as big,
            tc.tile_pool(name="pp", bufs=4) as pp,
            tc.tile_pool(name="sm", bufs=3) as sm,
            tc.tile_pool(name="ps", bufs=2, space="PSUM") as ps,
        ):
            mask = const.tile([128, 128], F32)
            make_upper_triangular(nc, mask[:], val=1.0, diag=True)
            qkb = const.tile([128, 4], F32)
            nc.sync.dma_start(out=qkb[:], in_=qkb_d.ap())
            vb = const.tile([128, FV], F32)
            nc.sync.dma_start(out=vb[:], in_=vb_d.ap())

            w_sb = big.tile([128, NDC * 768], F32R)
            for dc in range(NDC):
                nc.sync.dma_start(
                    out=w_sb[:, dc * 768 : (dc + 1) * 768],
                    in_=w_d.ap()[dc * 128 : (dc + 1) * 128, :],
                )
            xT_sb = big.tile([128, NDC * S], F32R)
            for sc in range(NQC):
                for dc in range(NDC):
                    nc.sync.dma_start(
                        out=xT_sb[:, dc * S + sc * 512 : dc * S + sc * 512 + 512],
                        in_=xT_d.ap()[dc * 128 : (dc + 1) * 128, sc * 512 : sc * 512 + 512],
                    )

            qkt = big.tile([128, 4 * S], F32R)  # rows: fc0..3 = Qh01,Qh23,Kh01,Kh23
            vcat = big.tile([128, NKB * HPC * VW], F32R)  # [V_h | 1] x 4 heads x kb
            nc.sync.dma_start(
                out=vcat.rearrange("p (k h j) -> p k h j", k=NKB, h=HPC)[
                    :, :, :, HD : HD + 1
                ],
                in_=ones_d.ap().rearrange("p (k h o) -> p k h o", k=NKB, o=1),
            )

            # ---- QKV projection ----
            for fc in range(4):  # QK.T row chunks
                for sc in range(NQC):
                    pt = ps.tile([128, 512], F32, tag="qkv", name=f"qk_{fc}_{sc}")
                    for dc in range(NDC):
                        nc.tensor.matmul(
                            pt[:],
                            w_sb[:, dc * 768 + fc * 128 : dc * 768 + fc * 128 + 128],
                            xT_sb[:, dc * S + sc * 512 : dc * S + sc * 512 + 512],
                            start=(dc == 0),
                            stop=(dc == NDC - 1),
                        )
                    nc.vector.tensor_scalar_add(
                        qkt[:, fc * S + sc * 512 : fc * S + sc * 512 + 512],
                        pt[:],
                        qkb[:, fc : fc + 1],
                    )
            for kc in range(NKB):  # V row chunks (128 of S)
                pt = ps.tile([128, 512], F32, tag="qkv", name=f"v_{kc}")
                for dc in range(NDC):
                    nc.tensor.matmul(
                        pt[:, 0:FV],
                        xT_sb[:, dc * S + kc * 128 : dc * S + kc * 128 + 128],
                        w_sb[:, dc * 768 + FQK : dc * 768 + 768],
                        start=(dc == 0),
                        stop=(dc == NDC - 1),
                    )
                nc.vector.tensor_tensor(
                    out=vcat.rearrange("p (k h j) -> p k h j", k=NKB, h=HPC)[
                        :, kc, :, 0:HD
                    ],
                    in0=pt[:, 0:FV].rearrange("p (h j) -> p h j", h=HPC),
                    in1=vb.rearrange("p (h j) -> p h j", h=HPC),
                    op=ADD,
                )

            # ---- attention, head pairs row-packed on the PE ----
            for pr in range(2):
                qoff = pr * S  # Q rows chunk fc=pr
                koff = (2 + pr) * S  # K rows chunk fc=2+pr
                for qc in range(NQC):
                    av = [
                        ps.tile([65, 512], F32, tag="av", name=f"av_{pr}_{qc}_{i}")
                        for i in (0, 1)
                    ]
                    nkb = 4 * qc + 4  # causal: k blocks 0 .. 4qc+3
                    for kb in range(nkb):
                        diag = kb // 4 == qc
                        off = 128 * (kb % 4) if diag else 0
                        st = ps.tile([128, 1024], F32, tag="st", name=f"st_{pr}_{qc}_{kb}")
                        for i in (0, 1):
                            nc.tensor.matmul(
                                st[:, i * 512 + off : i * 512 + 512],
                                qkt[64 * i : 64 * i + 64, koff + kb * 128 : koff + kb * 128 + 128],
                                qkt[64 * i : 64 * i + 64, qoff + qc * 512 + off : qoff + qc * 512 + 512],
                                start=True,
                                stop=True,
                            )
                        p_t = pp.tile([128, 1024], F32R, tag="p", name=f"p_{pr}_{qc}_{kb}")
                        nc.scalar.activation(
                            p_t.rearrange("p (h q) -> p h q", h=2)[:, :, off:512],
                            st.rearrange("p (h q) -> p h q", h=2)[:, :, off:512],
                            EXP,
                            scale=0.125,
                        )
                        if diag:
                            for i in (0, 1):
                                sl = p_t[:, i * 512 + off : i * 512 + off + 128]
                                nc.vector.tensor_tensor(out=sl, in0=sl, in1=mask[:], op=MULT)
                        for i in (0, 1):
                            h = 2 * pr + i
                            nc.tensor.matmul(
                                av[i][:, off:512],
                                vcat[:, (kb * HPC + h) * VW : (kb * HPC + h) * VW + VW],
                                p_t[:, i * 512 + off : i * 512 + 512],
                                start=(kb == 0),
                                stop=(kb == nkb - 1),
                            )
                    for i in (0, 1):
                        h = 2 * pr + i
                        r_t = sm.tile([1, 512], F32, tag="r", name=f"r_{pr}_{qc}_{i}")
                        nc.vector.reciprocal(r_t[:], av[i][64:65, :])
                        rr = sm.tile([64, 512], F32, tag="rr", name=f"rr_{pr}_{qc}_{i}")
                        nc.gpsimd.partition_broadcast(rr[:], r_t[:])
                        o_t = sm.tile([64, 512], F32, tag="o", name=f"o_{pr}_{qc}_{i}")
                        nc.vector.tensor_tensor(
                            out=o_t[:], in0=av[i][0:64, :], in1=rr[:], op=MULT
                        )
                        nc.sync.dma_start(
                            out=outT_d.ap()[64 * h : 64 * h + 64, qc * 512 : qc * 512 + 512],
                            in_=o_t[:],
                        )
    nc.compile()
    return nc


def round_fp32r(a):
    """Round fp32 to the fp32r grid (11-bit mantissa, low 12 bits zero),
    round-to-nearest-even — matches walrus's fp32_to_fp32r."""
    b = np.ascontiguousarray(a, dtype=np.float32).view(np.uint32)
    lsb = (b >> 12) & 1
    out = (b + 0x7FF + lsb) & 0xFFFFF000
    return out.view(np.float32)


def shard_inputs(x, W_qkv, b_qkv):
    """Full inputs -> list of 8 per-core input maps."""
    in_maps = []
    for c in range(N_CORES):
        b = c // (N_CORES // B)
        g = c % (N_CORES // B)
        heads = range(HPC * g, HPC * g + HPC)
        qcols = [h * 192 + j for h in heads for j in range(64)]
        kcols = [h * 192 + 64 + j for h in heads for j in range(64)]
        vcols = [h * 192 + 128 + j for h in heads for j in range(64)]
        cols = qcols + kcols + vcols
        w_sh = np.ascontiguousarray(W_qkv[:, cols], dtype=np.float32)
        b_sh = np.ascontiguousarray(b_qkv[cols], dtype=np.float32)
        qkb = np.ascontiguousarray(b_sh[:FQK].reshape(4, 128).T, dtype=np.float32)
        vb = np.ascontiguousarray(
            np.broadcast_to(b_sh[FQK:], (128, FV)), dtype=np.float32
        )
        xT = round_fp32r(np.ascontiguousarray(x[b].T, dtype=np.float32))
        ones = np.ones((128, S // 128 * HPC), dtype=np.float32)
        in_maps.append(
            {"xT": xT, "w": round_fp32r(w_sh), "qkb": qkb, "vb": vb, "ones": ones}
        )
    return in_maps


def gather_outputs(results):
    """8 per-core outT [256, S] -> full [B, S, D_H]."""
    out = np.empty((B, S, N_HEAD * HD), dtype=np.float32)
    for c in range(N_CORES):
        b = c // (N_CORES // B)
        g = c % (N_CORES // B)
        out[b, :, FV * g : FV * (g + 1)] = results[c]["outT"].T
    return out


_NC_CACHE = {}


def _get_nc():
    if "nc" not in _NC_CACHE:
        _NC_CACHE["nc"] = build_mha_core()
    return _NC_CACHE["nc"]


def kernel(x, W_qkv, b_qkv, _trace=False, _trace_kwargs=None):
    x = np.asarray(x, dtype=np.float32)
    W_qkv = np.asarray(W_qkv, dtype=np.float32)
    b_qkv = np.asarray(b_qkv, dtype=np.float32)
    nc = _get_nc()
    in_maps = shard_inputs(x, W_qkv, b_qkv)
    res = run_bass_kernel_spmd(
        nc, in_maps, list(range(N_CORES)), trace=_trace, **(_trace_kwargs or {})
    )
    out = gather_outputs(res.results)
    if _trace:
        kernel.last_results = res
    return out


# revision 14
# speedup vs baseline: 1.1821x; 1.1821x over previous
"""Causal multi-head attention (fused QKV) on 8 Trainium2 NeuronCores.

Problem: x[2, 2048, 1024] @ W_qkv[1024, 3072] -> causal MHA, 16 heads,
head_dim 64 -> out [2, 2048, 1024].

Sharding: batch (2) x head-groups (4) = 8 shards; core c handles batch
c//4, heads 4*(c%4) .. 4*(c%4)+3.  Each core is fully independent (no
collectives).

Per-core layouts (host prepares):
  xT   [D, S]      x[b].T  (contraction dim D on partitions)
  w    [D, 768]    W columns reordered [Q(4x64) | K(4x64) | V(4x64)]
  qkb  [128, 4]    QK bias per 128-row chunk (column fc)
  vb   [128, 256]  V bias replicated across partitions
  outT [256, S]    output transposed: row 64*h+j, col s = out[b,s,h,j]

On-core algorithm (all matmuls fp32r = full-rate fp32-reduced):
  QK.T = w_qk.T @ x.T   -> [512, S] rows (Q_h0|Q_h1|Q_h2|Q_h3|K_h0..)
  V    = x @ w_v        -> [S, 256], stored as [V_h|1] blocks of 65 cols
  per head pair, per q-chunk (512), per k-block (128, causal-pruned):
    S_T[k,q] = K@Q.T via 2 row-tiled (K=64) concurrent matmuls
    P = exp(S_T/8) on ScalarE (PSUM->SBUF, valid q range only)
    diagonal k-block: P *= upper-tri mask (DVE)
    out.T[hd+1, q] += [V|1].T @ P  (65th row accumulates softmax denom)
  normalize: r = 1/denom (DVE), partition-broadcast (GPSIMD), multiply.
"""

import sys

if "/opt/trn_rl_repo" not in sys.path:
    sys.path.insert(0, "/opt/trn_rl_repo")

import numpy as np

import concourse.bass as bass
import concourse.mybir as mybir
import concourse.tile as tile
from concourse import bacc
from concourse.bass_utils import run_bass_kernel_spmd
from concourse.masks import make_upper_triangular

F32 = mybir.dt.float32
F32R = mybir.dt.float32r
EXP = mybir.ActivationFunctionType.Exp
MULT = mybir.AluOpType.mult
ADD = mybir.AluOpType.add

N_CORES = 8
B, S, D = 2, 2048, 1024
N_HEAD = 16
HD = 64  # head dim
HPC = 4  # heads per core
FQK = 2 * HPC * HD  # 512 rows of QK.T
FV = HPC * HD  # 256 cols of V


def build_mha_core(S=S, D=D, trace_sim=False):
    """Build the per-core Bass program. Returns (nc, input names)."""
    NQC = S // 512  # 512-wide q chunks
    NKB = S // 128  # 128-wide k blocks
    NDC = D // 128  # 128-deep contraction chunks
    VW = HD + 1  # V block width incl. ones column

    nc = bacc.Bacc("TRN2", target_bir_lowering=False, debug=False)
    xT_d = nc.dram_tensor("xT", [D, S], F32R, kind="ExternalInput")
    w_d = nc.dram_tensor("w", [D, 768], F32R, kind="ExternalInput")
    qkb_d = nc.dram_tensor("qkb", [128, 4], F32, kind="ExternalInput")
    ones_d = nc.dram_tensor("ones", [128, S // 128 * HPC], F32R, kind="ExternalInput")
    vb_d = nc.dram_tensor("vb", [128, FV], F32, kind="ExternalInput")
    outT_d = nc.dram_tensor("outT", [FV, S], F32, kind="ExternalOutput")

    with tile.TileContext(nc, trace_sim=trace_sim) as tc:
        with (
            tc.tile_pool(name="const", bufs=1) as const,
            tc.tile_pool(name="big", bufs=1) as big,
            tc.tile_pool(name="pp", bufs=4) as pp,
            tc.tile_pool(name="sm", bufs=3) as sm,
            tc.tile_pool(name="ps", bufs=2, space="PSUM") as ps,
        ):
            mask = const.tile([128, 128], F32)
            make_upper_triangular(nc, mask[:], val=1.0, diag=True)
            qkb = const.tile([128, 4], F32)
            nc.sync.dma_start(out=qkb[:], in_=qkb_d.ap())
            vb = const.tile([128, FV], F32)
            nc.sync.dma_start(out=vb[:], in_=vb_d.ap())

            w_sb = big.tile([128, NDC * 768], F32R)
            for dc in range(NDC):
                nc.sync.dma_start(
                    out=w_sb[:, dc * 768 : (dc + 1) * 768],
                    in_=w_d.ap()[dc * 128 : (dc + 1) * 128, :],
                )
            xT_sb = big.tile([128, NDC * S], F32R)
            for sc in range(NQC):
                for dc in range(NDC):
                    nc.sync.dma_start(
                        out=xT_sb[:, dc * S + sc * 512 : dc * S + sc * 512 + 512],
                        in_=xT_d.ap()[dc * 128 : (dc + 1) * 128, sc * 512 : sc * 512 + 512],
                    )

            qkt = big.tile([128, 4 * S], F32R)  # rows: fc0..3 = Qh01,Qh23,Kh01,Kh23
            vcat = big.tile([128, NKB * HPC * VW], F32R)  # [V_h | 1] x 4 heads x kb
            nc.sync.dma_start(
                out=vcat.rearrange("p (k h j) -> p k h j", k=NKB, h=HPC)[
                    :, :, :, HD : HD + 1
                ],
                in_=ones_d.ap().rearrange("p (k h o) -> p k h o", k=NKB, o=1),
            )

            # ---- QKV projection ----
            for fc in range(4):  # QK.T row chunks
                for sc in range(NQC):
                    pt = ps.tile([128, 512], F32, tag="st", name=f"qk_{fc}_{sc}")
                    for dc in range(NDC):
                        nc.tensor.matmul(
                            pt[:],
                            w_sb[:, dc * 768 + fc * 128 : dc * 768 + fc * 128 + 128],
                            xT_sb[:, dc * S + sc * 512 : dc * S + sc * 512 + 512],
                            start=(dc == 0),
                            stop=(dc == NDC - 1),
                        )
                    nc.vector.tensor_scalar_add(
                        qkt[:, fc * S + sc * 512 : fc * S + sc * 512 + 512],
                        pt[:],
                        qkb[:, fc : fc + 1],
                    )
            for kc in range(NKB):  # V row chunks (128 of S)
                pt = ps.tile([128, 512], F32, tag="st", name=f"v_{kc}")
                for dc in range(NDC):
                    nc.tensor.matmul(
                        pt[:, 0:FV],
                        xT_sb[:, dc * S + kc * 128 : dc * S + kc * 128 + 128],
                        w_sb[:, dc * 768 + FQK : dc * 768 + 768],
                        start=(dc == 0),
                        stop=(dc == NDC - 1),
                    )
                nc.vector.tensor_tensor(
                    out=vcat.rearrange("p (k h j) -> p k h j", k=NKB, h=HPC)[
                        :, kc, :, 0:HD
                    ],
                    in0=pt[:, 0:FV].rearrange("p (h j) -> p h j", h=HPC),
                    in1=vb.rearrange("p (h j) -> p h j", h=HPC),
                    op=ADD,
                )

            # ---- attention, head pairs row-packed on the PE ----
            for pr in range(2):
                qoff = pr * S  # Q rows chunk fc=pr
                koff = (2 + pr) * S  # K rows chunk fc=2+pr
                for qc in range(NQC):
                    av = [
                        ps.tile([65, 512], F32, tag="av", bufs=4, name=f"av_{pr}_{qc}_{i}")
                        for i in (0, 1)
                    ]
                    nkb = 4 * qc + 4  # causal: k blocks 0 .. 4qc+3
                    for kb in range(nkb):
                        diag = kb // 4 == qc
                        off = 128 * (kb % 4) if diag else 0
                        st = ps.tile([128, 1024], F32, tag="st", name=f"st_{pr}_{qc}_{kb}")
                        for i in (0, 1):
                            nc.tensor.matmul(
                                st[:, i * 512 + off : i * 512 + 512],
                                qkt[64 * i : 64 * i + 64, koff + kb * 128 : koff + kb * 128 + 128],
                                qkt[64 * i : 64 * i + 64, qoff + qc * 512 + off : qoff + qc * 512 + 512],
                                start=True,
                                stop=True,
                            )
                        p_t = pp.tile([128, 1024], F32R, tag="p", name=f"p_{pr}_{qc}_{kb}")
                        nc.scalar.activation(
                            p_t.rearrange("p (h q) -> p h q", h=2)[:, :, off:512],
                            st.rearrange("p (h q) -> p h q", h=2)[:, :, off:512],
                            EXP,
                            scale=0.125,
                        )
                        if diag:
                            for i in (0, 1):
                                sl = p_t[:, i * 512 + off : i * 512 + off + 128]
                                nc.vector.tensor_tensor(out=sl, in0=sl, in1=mask[:], op=MULT)
                        for i in (0, 1):
                            h = 2 * pr + i
                            nc.tensor.matmul(
                                av[i][:, off:512],
                                vcat[:, (kb * HPC + h) * VW : (kb * HPC + h) * VW + VW],
                                p_t[:, i * 512 + off : i * 512 + 512],
                                start=(kb == 0),
                                stop=(kb == nkb - 1),
                            )
                    for i in (0, 1):
                        h = 2 * pr + i
                        s_t = sm.tile([1, 512], F32, tag="r", name=f"s_{pr}_{qc}_{i}")
                        nc.vector.tensor_copy(out=s_t[:], in_=av[i][64:65, :])
                        ss = sm.tile([64, 512], F32, tag="ss", name=f"ss_{pr}_{qc}_{i}")
                        nc.gpsimd.partition_broadcast(ss[:], s_t[:])
                        rr = sm.tile([64, 512], F32, tag="rr", name=f"rr_{pr}_{qc}_{i}")
                        nc.vector.reciprocal_approx_fast(rr[:], ss[:])
                        o_t = sm.tile([64, 512], F32, tag="o", name=f"o_{pr}_{qc}_{i}")
                        nc.vector.tensor_tensor(
                            out=o_t[:], in0=av[i][0:64, :], in1=rr[:], op=MULT
                        )
                        nc.sync.dma_start(
                            out=outT_d.ap()[64 * h : 64 * h + 64, qc * 512 : qc * 512 + 512],
                            in_=o_t[:],
                        )
    nc.compile()
    return nc


def round_fp32r(a):
    """Round fp32 to the fp32r grid (11-bit mantissa, low 12 bits zero),
    round-to-nearest-even — matches walrus's fp32_to_fp32r."""
    b = np.ascontiguousarray(a, dtype=np.float32).view(np.uint32)
    lsb = (b >> 12) & 1
    out = (b + 0x7FF + lsb) & 0xFFFFF000
    return out.view(np.float32)


def shard_inputs(x, W_qkv, b_qkv):
    """Full inputs -> list of 8 per-core input maps."""
    in_maps = []
    for c in range(N_CORES):
        b = c // (N_CORES // B)
        g = c % (N_CORES // B)
        heads = range(HPC * g, HPC * g + HPC)
        qcols = [h * 192 + j for h in heads for j in range(64)]
        kcols = [h * 192 + 64 + j for h in heads for j in range(64)]
        vcols = [h * 192 + 128 + j for h in heads for j in range(64)]
        cols = qcols + kcols + vcols
        w_sh = np.ascontiguousarray(W_qkv[:, cols], dtype=np.float32)
        b_sh = np.ascontiguousarray(b_qkv[cols], dtype=np.float32)
        qkb = np.ascontiguousarray(b_sh[:FQK].reshape(4, 128).T, dtype=np.float32)
        vb = np.ascontiguousarray(
            np.broadcast_to(b_sh[FQK:], (128, FV)), dtype=np.float32
        )
        xT = round_fp32r(np.ascontiguousarray(x[b].T, dtype=np.float32))
        ones = np.ones((128, S // 128 * HPC), dtype=np.float32)
        in_maps.append(
            {"xT": xT, "w": round_fp32r(w_sh), "qkb": qkb, "vb": vb, "ones": ones}
        )
    return in_maps


def gather_outputs(results):
    """8 per-core outT [256, S] -> full [B, S, D_H]."""
    out = np.empty((B, S, N_HEAD * HD), dtype=np.float32)
    for c in range(N_CORES):
        b = c // (N_CORES // B)
        g = c % (N_CORES // B)
        out[b, :, FV * g : FV * (g + 1)] = results[c]["outT"].T
    return out


_NC_CACHE = {}


def _get_nc():
    if "nc" not in _NC_CACHE:
        _NC_CACHE["nc"] = build_mha_core()
    return _NC_CACHE["nc"]


def kernel(x, W_qkv, b_qkv, _trace=False, _trace_kwargs=None):
    x = np.asarray(x, dtype=np.float32)
    W_qkv = np.asarray(W_qkv, dtype=np.float32)
    b_qkv = np.asarray(b_qkv, dtype=np.float32)
    nc = _get_nc()
    in_maps = shard_inputs(x, W_qkv, b_qkv)
    res = run_bass_kernel_spmd(
        nc, in_maps, list(range(N_CORES)), trace=_trace, **(_trace_kwargs or {})
    )
    out = gather_outputs(res.results)
    if _trace:
        kernel.last_results = res
    return out


# revision 20
# speedup vs baseline: 1.3212x; 1.1177x over previous
"""Causal multi-head attention (fused QKV) on 8 Trainium2 NeuronCores.

Problem: x[2, 2048, 1024] @ W_qkv[1024, 3072] -> causal MHA, 16 heads,
head_dim 64 -> out [2, 2048, 1024].

Sharding: batch (2) x head-groups (4) = 8 shards; core c handles batch
c//4, heads 4*(c%4) .. 4*(c%4)+3.  Each core is fully independent (no
collectives).

Per-core layouts (host prepares):
  xT   [D, S]      x[b].T  (contraction dim D on partitions)
  w    [D, 768]    W columns reordered [Q(4x64) | K(4x64) | V(4x64)]
  qkb  [128, 4]    QK bias per 128-row chunk (column fc)
  vb   [128, 256]  V bias replicated across partitions
  outT [256, S]    output transposed: row 64*h+j, col s = out[b,s,h,j]

On-core algorithm (all matmuls fp32r = full-rate fp32-reduced):
  QK.T = w_qk.T @ x.T   -> [512, S] rows (Q_h0|Q_h1|Q_h2|Q_h3|K_h0..)
  V    = x @ w_v        -> [S, 256], stored as [V_h|1] blocks of 65 cols
  per head pair, per q-chunk (512), per k-block (128, causal-pruned):
    S_T[k,q] = K@Q.T via 2 row-tiled (K=64) concurrent matmuls
    P = exp(S_T/8) on ScalarE (PSUM->SBUF, valid q range only)
    diagonal k-block: P *= upper-tri mask (DVE)
    out.T[hd+1, q] += [V|1].T @ P  (65th row accumulates softmax denom)
  normalize: r = 1/denom (DVE), partition-broadcast (GPSIMD), multiply.
"""

import sys

if "/opt/trn_rl_repo" not in sys.path:
    sys.path.insert(0, "/opt/trn_rl_repo")

import numpy as np
import ml_dtypes

import concourse.bass as bass
import concourse.mybir as mybir
import concourse.tile as tile
from concourse import bacc
from concourse.bass_utils import run_bass_kernel_spmd
from concourse.masks import make_upper_triangular

F32 = mybir.dt.float32
F32R = mybir.dt.float32r
BF16 = mybir.dt.bfloat16
ATTN_DT = BF16  # dtype of Q/K/V/P inside attention (matmul operands)
EXP = mybir.ActivationFunctionType.Exp
MULT = mybir.AluOpType.mult
ADD = mybir.AluOpType.add

N_CORES = 8
B, S, D = 2, 2048, 1024
N_HEAD = 16
HD = 64  # head dim
HPC = 4  # heads per core
FQK = 2 * HPC * HD  # 512 rows of QK.T
FV = HPC * HD  # 256 cols of V


def build_mha_core(S=S, D=D, trace_sim=False, debug_taps=False):
    """Build the per-core Bass program. Returns (nc, input names)."""
    NQC = S // 512  # 512-wide q chunks
    NKB = S // 128  # 128-wide k blocks
    NDC = D // 128  # 128-deep contraction chunks
    VW = HD + 1  # V block width incl. ones column

    nc = bacc.Bacc("TRN2", target_bir_lowering=False, debug=False)
    xT_d = nc.dram_tensor("xT", [D, S], F32R, kind="ExternalInput")
    w_d = nc.dram_tensor("w", [D, 768], F32R, kind="ExternalInput")
    qkb_d = nc.dram_tensor("qkb", [128, 4], F32, kind="ExternalInput")
    ones_d = nc.dram_tensor("ones", [128, S // 128 * HPC], F32, kind="ExternalInput")
    vb_d = nc.dram_tensor("vb", [128, FV], F32, kind="ExternalInput")
    outT_d = nc.dram_tensor("outT", [FV, S], F32, kind="ExternalOutput")
    if debug_taps:
        dbg_qkt = nc.dram_tensor("dbg_qkt", [128, 4 * S], ATTN_DT, kind="ExternalOutput")
        dbg_vcat = nc.dram_tensor(
            "dbg_vcat", [128, (S // 128) * HPC * (HD + 1)], ATTN_DT, kind="ExternalOutput"
        )

    with tile.TileContext(nc, trace_sim=trace_sim) as tc:
        with (
            tc.tile_pool(name="const", bufs=1) as const,
            tc.tile_pool(name="big", bufs=1) as big,
            tc.tile_pool(name="pp", bufs=4) as pp,
            tc.tile_pool(name="sm", bufs=3) as sm,
            tc.tile_pool(name="ps", bufs=2, space="PSUM") as ps,
        ):
            mask = const.tile([128, 128], F32)
            make_upper_triangular(nc, mask[:], val=1.0, diag=True)
            qkb = const.tile([128, 4], F32)
            nc.sync.dma_start(out=qkb[:], in_=qkb_d.ap())
            vb = const.tile([128, FV], F32)
            nc.sync.dma_start(out=vb[:], in_=vb_d.ap())

            w_sb = big.tile([128, NDC * 768], F32R)
            for dc in range(NDC):
                nc.sync.dma_start(
                    out=w_sb[:, dc * 768 : (dc + 1) * 768],
                    in_=w_d.ap()[dc * 128 : (dc + 1) * 128, :],
                )
            xT_sb = big.tile([128, NDC * S], F32R)
            for sc in range(NQC):
                for dc in range(NDC):
                    nc.sync.dma_start(
                        out=xT_sb[:, dc * S + sc * 512 : dc * S + sc * 512 + 512],
                        in_=xT_d.ap()[dc * 128 : (dc + 1) * 128, sc * 512 : sc * 512 + 512],
                    )

            qkt = big.tile([128, 4 * S], ATTN_DT)  # rows: fc0..3 = Qh01,Qh23,Kh01,Kh23
            vcat = big.tile([128, NKB * HPC * VW], ATTN_DT)  # [V_h | 1] x 4 heads x kb
            ones_sb = const.tile([128, NKB * HPC], F32)
            nc.sync.dma_start(out=ones_sb[:], in_=ones_d.ap())
            nc.vector.tensor_copy(
                out=vcat.rearrange("p (k h j) -> p k h j", k=NKB, h=HPC)[
                    :, :, :, HD : HD + 1
                ],
                in_=ones_sb.rearrange("p (k h o) -> p k h o", k=NKB, o=1),
            )

            # ---- QKV projection ----
            for fc in range(4):  # QK.T row chunks
                for sc in range(NQC):
                    pt = ps.tile([128, 512], F32, tag="st", name=f"qk_{fc}_{sc}")
                    for dc in range(NDC):
                        nc.tensor.matmul(
                            pt[:],
                            w_sb[:, dc * 768 + fc * 128 : dc * 768 + fc * 128 + 128],
                            xT_sb[:, dc * S + sc * 512 : dc * S + sc * 512 + 512],
                            start=(dc == 0),
                            stop=(dc == NDC - 1),
                        )
                    nc.vector.tensor_scalar_add(
                        qkt[:, fc * S + sc * 512 : fc * S + sc * 512 + 512],
                        pt[:],
                        qkb[:, fc : fc + 1],
                    )
            for kc in range(NKB):  # V row chunks (128 of S)
                pt = ps.tile([128, 512], F32, tag="st", name=f"v_{kc}")
                for dc in range(NDC):
                    nc.tensor.matmul(
                        pt[:, 0:FV],
                        xT_sb[:, dc * S + kc * 128 : dc * S + kc * 128 + 128],
                        w_sb[:, dc * 768 + FQK : dc * 768 + 768],
                        start=(dc == 0),
                        stop=(dc == NDC - 1),
                    )
                nc.vector.tensor_tensor(
                    out=vcat.rearrange("p (k h j) -> p k h j", k=NKB, h=HPC)[
                        :, kc, :, 0:HD
                    ],
                    in0=pt[:, 0:FV].rearrange("p (h j) -> p h j", h=HPC),
                    in1=vb.rearrange("p (h j) -> p h j", h=HPC),
                    op=ADD,
                )

            # ---- attention, head pairs row-packed on the PE ----
            for pr in range(2):
                qoff = pr * S  # Q rows chunk fc=pr
                koff = (2 + pr) * S  # K rows chunk fc=2+pr
                for qc in (range(NQC) if pr == 0 else reversed(range(NQC))):
                    av = [
                        ps.tile([65, 512], F32, tag="av", bufs=4, name=f"av_{pr}_{qc}_{i}")
                        for i in (0, 1)
                    ]
                    nkb = 4 * qc + 4  # causal: k blocks 0 .. 4qc+3
                    for kb in range(nkb):
                        diag = kb // 4 == qc
                        off = 128 * (kb % 4) if diag else 0
                        st = ps.tile([128, 1024], F32, tag="st", name=f"st_{pr}_{qc}_{kb}")
                        for i in (0, 1):
                            nc.tensor.matmul(
                                st[:, i * 512 + off : i * 512 + 512],
                                qkt[64 * i : 64 * i + 64, koff + kb * 128 : koff + kb * 128 + 128],
                                qkt[64 * i : 64 * i + 64, qoff + qc * 512 + off : qoff + qc * 512 + 512],
                                start=True,
                                stop=True,
                            )
                        p_t = pp.tile([128, 1024], ATTN_DT, tag="p", name=f"p_{pr}_{qc}_{kb}")
                        nc.scalar.activation(
                            p_t.rearrange("p (h q) -> p h q", h=2)[:, :, off:512],
                            st.rearrange("p (h q) -> p h q", h=2)[:, :, off:512],
                            EXP,
                            scale=0.125,
                        )
                        if diag:
                            for i in (0, 1):
                                sl = p_t[:, i * 512 + off : i * 512 + off + 128]
                                nc.vector.tensor_tensor(out=sl, in0=sl, in1=mask[:], op=MULT)
                        for i in (0, 1):
                            h = 2 * pr + i
                            nc.tensor.matmul(
                                av[i][:, off:512],
                                vcat[:, (kb * HPC + h) * VW : (kb * HPC + h) * VW + VW],
                                p_t[:, i * 512 + off : i * 512 + 512],
                                start=(kb == 0),
                                stop=(kb == nkb - 1),
                            )
                    for i in (0, 1):
                        h = 2 * pr + i
                        ou = sm.tile([65, 512], F32, tag="ou", name=f"ou_{pr}_{qc}_{i}")
                        nc.vector.tensor_copy(out=ou[:], in_=av[i][:])
                        s_t = sm.tile([1, 512], F32, tag="s", name=f"s_{pr}_{qc}_{i}")
                        nc.vector.tensor_copy(out=s_t[:], in_=ou[64:65, :])
                        ss = sm.tile([64, 512], F32, tag="ss", name=f"ss_{pr}_{qc}_{i}")
                        nc.gpsimd.partition_broadcast(ss[:], s_t[:])
                        rr = sm.tile([64, 512], F32, tag="rr", name=f"rr_{pr}_{qc}_{i}")
                        nc.vector.reciprocal_approx_fast(rr[:], ss[:])
                        nc.vector.tensor_tensor(
                            out=ou[0:64, :], in0=ou[0:64, :], in1=rr[:], op=MULT
                        )
                        nc.sync.dma_start(
                            out=outT_d.ap()[64 * h : 64 * h + 64, qc * 512 : qc * 512 + 512],
                            in_=ou[0:64, :],
                        )
            if debug_taps:
                nc.sync.dma_start(out=dbg_qkt.ap(), in_=qkt[:])
                nc.sync.dma_start(out=dbg_vcat.ap(), in_=vcat[:])
    nc.compile()
    return nc


def round_fp32r(a):
    """Round fp32 to the fp32r grid (11-bit mantissa, low 12 bits zero),
    round-to-nearest-even — matches walrus's fp32_to_fp32r."""
    b = np.ascontiguousarray(a, dtype=np.float32).view(np.uint32)
    lsb = (b >> 12) & 1
    out = (b + 0x7FF + lsb) & 0xFFFFF000
    return out.view(np.float32)


def shard_inputs(x, W_qkv, b_qkv):
    """Full inputs -> list of 8 per-core input maps."""
    in_maps = []
    for c in range(N_CORES):
        b = c // (N_CORES // B)
        g = c % (N_CORES // B)
        heads = range(HPC * g, HPC * g + HPC)
        qcols = [h * 192 + j for h in heads for j in range(64)]
        kcols = [h * 192 + 64 + j for h in heads for j in range(64)]
        vcols = [h * 192 + 128 + j for h in heads for j in range(64)]
        cols = qcols + kcols + vcols
        w_sh = np.ascontiguousarray(W_qkv[:, cols], dtype=np.float32)
        b_sh = np.ascontiguousarray(b_qkv[cols], dtype=np.float32)
        qkb = np.ascontiguousarray(b_sh[:FQK].reshape(4, 128).T, dtype=np.float32)
        vb = np.ascontiguousarray(
            np.broadcast_to(b_sh[FQK:], (128, FV)), dtype=np.float32
        )
        xT = round_fp32r(np.ascontiguousarray(x[b].T, dtype=np.float32))
        ones = np.ones((128, S // 128 * HPC), dtype=np.float32)
        in_maps.append(
            {"xT": xT, "w": round_fp32r(w_sh), "qkb": qkb, "vb": vb, "ones": ones}
        )
    return in_maps


def gather_outputs(results):
    """8 per-core outT [256, S] -> full [B, S, D_H]."""
    out = np.empty((B, S, N_HEAD * HD), dtype=np.float32)
    for c in range(N_CORES):
        b = c // (N_CORES // B)
        g = c % (N_CORES // B)
        out[b, :, FV * g : FV * (g + 1)] = results[c]["outT"].T
    return out


_NC_CACHE = {}


def _get_nc():
    if "nc" not in _NC_CACHE:
        _NC_CACHE["nc"] = build_mha_core()
    return _NC_CACHE["nc"]


def kernel(x, W_qkv, b_qkv, _trace=False, _trace_kwargs=None):
    x = np.asarray(x, dtype=np.float32)
    W_qkv = np.asarray(W_qkv, dtype=np.float32)
    b_qkv = np.asarray(b_qkv, dtype=np.float32)
    nc = _get_nc()
    in_maps = shard_inputs(x, W_qkv, b_qkv)
    res = run_bass_kernel_spmd(
        nc, in_maps, list(range(N_CORES)), trace=_trace, **(_trace_kwargs or {})
    )
    out = gather_outputs(res.results)
    if _trace:
        kernel.last_results = res
    return out
